# revision 1
# baseline (speedup 1.0000x reference)
"""STFT magnitude spectrogram kernel for Trainium2 (8 NeuronCores).

Computes, for x (64, 160000):
  out[b, k, t] = |sum_n w[n] * x[b, 256*t + n] * exp(-2i*pi*k*n/1024)|
with w the normalized (fractionally-shifted) Hann window from the
reference. Data-parallel over batch: 8 rows per core.

Device algorithm per core (8 batch rows):
  1. DMA x rows into SBUF in natural layout (chunk-of-256 on partitions).
  2. PE-transpose into two "streams" S_h[p, u] = x[256*u + 128*h + p]
     (sample-offset on partitions).  All 8 contraction chunks of every
     frame are column-shifted views of these two streams, so x is read
     from HBM exactly once.
  3. Window-folded DFT: out(f, t) tiles = sum_c CW[c]^T @ S view, as
     float32r matmuls accumulated over 8 chunks of 128 in PSUM.
  4. Magnitude: re^2 (ScalarE) + im^2 (ScalarE), add (VectorE),
     sqrt (ScalarE) -> SBUF -> DMA to out.
"""

import sys

sys.path.insert(0, "/opt/trn_rl_repo")

import numpy as np

N = 1024
STRIDE = 256
B = 64
L = 160000
T = 622          # frames
F = 513          # rfft bins
NCORES = 8
BPC = B // NCORES  # batch rows per core
NCH = N // 128     # 8 contraction chunks
NU = L // STRIDE + 1  # 625 stream columns (624*256+127+128 = 159999 max index)
TSPLIT = (312, 310)  # frame tiles: even widths (fp32r needs even moving dim), >=256
NJ = 5             # 128-wide chunk blocks per row (625 = 4*128 + 113)
JTAIL = 625 - 4 * 128  # 113

_prog_cache = {}


def _patch_fast_compile():
    """Disable the BIR simulator inside walrus codegen: it is only a
    verification aid and costs ~50 min on this kernel (vs ~3 min off)."""
    import concourse.bass_utils as bu

    if getattr(bu, "_fast_compile_patched", False):
        return
    from pathlib import Path

    from concourse.aot_env import aot_getenv

    def bir_verify_and_optimise(
        tmpdir, inp="bir.json", outp="file.neff", arch=None, *, dve_root=None
    ):
        cmd = [
            bu.get_walrus_driver(),
            "--pass",
            ",".join(
                [
                    "birverifier",
                    "runtime_memory_reservation",
                    "lower_act",
                    "lower_dve",
                    "lower_ap_offset",
                    "codegen",
                    "neff_packager",
                ]
            ),
            "-i", inp,
            "--neff-output-filename", outp,
            "--enable-birsim=false",
            "--mem-mode=physical",
            "--policy=0",
            "--enable-ldw-opt=false",
            "--assign-static-dmas-to-sp=false",
            f"--dram-page-size={aot_getenv('NEURON_SCRATCHPAD_PAGE_SIZE', '256')}",
            "--enable-neff-debug-info=true",
            "--jobs", "8",
            *bu.get_walrus_args(
                bu.get_bir_arch(tmpdir, inp) if arch is None else arch,
                tmpdir,
                dve_root=dve_root,
            ),
        ]
        result = bu.run_command(cmd, cwd=tmpdir)
        if result is not None:
            (Path(tmpdir) / "log.txt").write_text(result.stdout)
        return f"{tmpdir}/{outp}"

    bu.bir_verify_and_optimise = bir_verify_and_optimise
    bu._fast_compile_patched = True


def _build_program():
    _patch_fast_compile()
    import concourse.bass as bass
    import concourse.mybir as mybir
    import concourse.tile as tile
    from concourse import bacc
    from concourse.masks import make_identity

    f32 = mybir.dt.float32
    f32r = mybir.dt.float32r

    nc = bacc.Bacc("TRN2", target_bir_lowering=False, enable_partition_id=False)

    xs = nc.dram_tensor("xs", [BPC, L], f32, kind="ExternalInput")
    cw = nc.dram_tensor("cw", [NCH, 128, F], f32, kind="ExternalInput")
    sw = nc.dram_tensor("sw", [NCH, 128, F], f32, kind="ExternalInput")
    out = nc.dram_tensor("out", [BPC, F, T], f32, kind="ExternalOutput")

    Square = mybir.ActivationFunctionType.Square
    Sqrt = mybir.ActivationFunctionType.Sqrt

    with tile.TileContext(nc) as tc:
        with (
            tc.tile_pool(name="const", bufs=1) as const_pool,
            tc.tile_pool(name="xn", bufs=2) as xn_pool,
            tc.tile_pool(name="streams", bufs=BPC) as stream_pool,
            tc.tile_pool(name="sq", bufs=3) as sq_pool,
            tc.tile_pool(name="outsb", bufs=3) as out_pool,
            tc.tile_pool(name="ptrans", bufs=2, space="PSUM") as pt_pool,
            tc.tile_pool(name="pmm", bufs=3, space="PSUM") as pmm_pool,
        ):
            ident = const_pool.tile([128, 128], f32)
            make_identity(nc, ident[:])

            # DFT matrices -> SBUF, rounded to float32r during the (SWDGE) DMA.
            cw_sb = const_pool.tile([128, NCH, F], f32r)
            sw_sb = const_pool.tile([128, NCH, F], f32r)
            for dram_m, sb_m in ((cw, cw_sb), (sw, sw_sb)):
                for c in range(NCH):
                    nc.gpsimd.dma_start(
                        sb_m[:, c, :], dram_m[c].rearrange("p k -> p k")
                    )

            streams = []  # [b][h] -> (128, NU) f32r
            for b in range(BPC):
                xn_main = xn_pool.tile([128, 4, 256], f32, tag="xn_main")
                xn_tail = xn_pool.tile([128, 256], f32, tag="xn_tail")
                nc.sync.dma_start(
                    xn_main[:],
                    xs[b, 0 : 4 * 128 * 256].rearrange(
                        "(j p r) -> p j r", j=4, p=128, r=256
                    ),
                )
                nc.sync.dma_start(
                    xn_tail[0:JTAIL, :],
                    xs[b, 4 * 128 * 256 : L].rearrange("(p r) -> p r", p=JTAIL),
                )
                s_pair = []
                for h in range(2):
                    s_h = stream_pool.tile([128, NU], f32r, tag=f"s{h}")
                    for j in range(NJ):
                        if j < 4:
                            src = xn_main[:, j, 128 * h : 128 * h + 128]
                            width = 128
                        else:
                            src = xn_tail[0:JTAIL, 128 * h : 128 * h + 128]
                            width = JTAIL
                        tp = pt_pool.tile([128, 128], f32, tag="tp")
                        nc.tensor.transpose(
                            tp[:, 0:width], src, ident[0:width, 0:width]
                        )
                        nc.vector.tensor_copy(
                            s_h[:, 128 * j : 128 * j + width], tp[:, 0:width]
                        )
                    s_pair.append(s_h)
                streams.append(s_pair)

            # Main DFT matmuls + magnitude.
            for b in range(BPC):
                for f in range(4):
                    o_sb = out_pool.tile([128, T], f32, tag="o_sb")
                    for ti in range(2):
                        t0 = ti * TSPLIT[0]
                        W = TSPLIT[ti]
                        p_re = pmm_pool.tile([128, W], f32, tag="p_re")
                        p_im = pmm_pool.tile([128, W], f32, tag="p_im")
                        for c in range(NCH):
                            rhs = streams[b][c & 1][:, (c >> 1) + t0 : (c >> 1) + t0 + W]
                            kw = dict(start=(c == 0), stop=(c == NCH - 1))
                            nc.tensor.matmul(
                                p_re[:], cw_sb[:, c, 128 * f : 128 * f + 128], rhs, **kw
                            )
                            nc.tensor.matmul(
                                p_im[:], sw_sb[:, c, 128 * f : 128 * f + 128], rhs, **kw
                            )
                        sq_re = sq_pool.tile([128, TSPLIT[0]], f32, tag="sq_re")
                        sq_im = sq_pool.tile([128, TSPLIT[0]], f32, tag="sq_im")
                        nc.scalar.activation(sq_re[:, 0:W], p_re[:], Square)
                        nc.scalar.activation(sq_im[:, 0:W], p_im[:], Square)
                        ssum = sq_pool.tile([128, TSPLIT[0]], f32, tag="ssum")
                        nc.vector.tensor_add(ssum[:, 0:W], sq_re[:, 0:W], sq_im[:, 0:W])
                        nc.scalar.activation(o_sb[:, t0 : t0 + W], ssum[:, 0:W], Sqrt)
                    nc.sync.dma_start(out[b, 128 * f : 128 * f + 128, :], o_sb[:])


    nc.compile()
    return nc


def _host_params(win_length, strides, win_pow):
    """Reproduce the reference's parameter transforms on the host."""
    wl = float(np.clip(np.asarray(win_length, np.float64)[0], N / 20.0, float(N)))
    st = float(np.clip(np.asarray(strides, np.float64)[0], 0.0, float(N)))

    es = np.full((T,), st, np.float64)
    frames = np.concatenate([[0.0], np.cumsum(es[1:])])
    idx_floor = np.floor(frames)
    idx_frac = frames - idx_floor

    if not (np.all(idx_frac == 0.0) and np.all(idx_floor == STRIDE * np.arange(T))):
        raise NotImplementedError(
            "kernel fast path requires integer frame stride of 256"
        )

    base = np.arange(N, dtype=np.float64)
    tap = 0.5 - 0.5 * np.cos(2.0 * np.pi * (base + (wl - N + 1) / 2.0) / wl)
    mask = (base >= np.ceil((N - 1 + wl) / 2.0)) | (base <= np.floor((N - 1 - wl) / 2.0))
    tap[mask] = 0.0
    tap = tap / tap.sum()
    tap = tap ** float(np.asarray(win_pow, np.float64)[0])
    return tap


def kernel(x, win_length, strides, win_pow):
    from concourse.bass_utils import run_bass_kernel_spmd

    x = np.ascontiguousarray(np.asarray(x, dtype=np.float32))
    assert x.shape == (B, L)

    tap = _host_params(win_length, strides, win_pow)

    n = np.arange(N, dtype=np.float64)
    k = np.arange(F, dtype=np.float64)
    ang = 2.0 * np.pi * np.outer(n, k) / N
    CW = (tap[:, None] * np.cos(ang)).astype(np.float32).reshape(NCH, 128, F)
    SW = (tap[:, None] * np.sin(ang)).astype(np.float32).reshape(NCH, 128, F)
    CW = np.ascontiguousarray(CW)
    SW = np.ascontiguousarray(SW)

    if "nc" not in _prog_cache:
        _prog_cache["nc"] = _build_program()
    nc = _prog_cache["nc"]

    in_maps = [
        {"xs": x[c * BPC : (c + 1) * BPC], "cw": CW, "sw": SW}
        for c in range(NCORES)
    ]
    res = run_bass_kernel_spmd(nc, in_maps, core_ids=list(range(NCORES)))
    outp = np.empty((B, F, T), dtype=np.float32)
    for c in range(NCORES):
        outp[c * BPC : (c + 1) * BPC] = res.results[c]["out"]

    # Nyquist row k=512 on host: X[512] = sum_n (-1)^n w[n] x[.,256t+n]
    wn = (tap * ((-1.0) ** np.arange(N))).astype(np.float32)
    frames_v = np.lib.stride_tricks.as_strided(
        x,
        shape=(B, T, N),
        strides=(x.strides[0], STRIDE * x.itemsize, x.itemsize),
    )
    outp[:, 512, :] = np.abs(frames_v @ wn)
    return outp

